# revision 2
# baseline (speedup 1.0000x reference)
"""Trainium2 Bass kernel: batched bilinear form  out[n] = elg[n] @ W @ eth[n].

Problem: elg, eth [32768, 1024] fp32, W [1024, 1024] fp32.
Sharding: data-parallel over the batch (N) axis across 8 NeuronCores;
W is replicated.  Per core (4096 rows):

    T      = elg @ W                   (TensorE)
    out[n] = sum_e T[n,e] * eth[n,e]   (VectorE fused multiply-reduce, fp32)

Precision/speed split of the contraction (d) axis:
  d in [0, 512):    fp16 x fp16 matmuls (1 cyc/row on the PE).
  d in [512, 1024): fp8 DoubleRow matmuls at 0.5 cyc/row.  The two
      DoubleRow slots hold (elg8, elg8) duplicated against (W_hi, W_lo),
      an e4m3 hi+lo split of W -- so W keeps ~14-bit precision and only
      elg pays one-sided e4m3 quantization error on half the volume.
      Measured rel err of the full pipeline vs the f64 reference: 1.94e-2
      (gate 2e-2); fp16-only would be 3.2e-4 at ~33% more PE time.

All operand layouts are packed host-side so every DMA is a large
fully-contiguous-per-partition transfer; no DMA transpose, no on-chip
transposes.  eth is fp16 (halves its HBM traffic; fp32 accumulate in DVE).
"""

import numpy as np

N_TOTAL = 32768
D = 1024
N_CORES = 8
N_CORE = N_TOTAL // N_CORES          # 4096 rows per core
P = 128                              # SBUF/PSUM partitions
KF = 4                               # fp16 k-tiles (d < 512)
KQ = 4                               # fp8 DoubleRow k-tiles (d >= 512)
D_SPLIT = KF * P                     # 512
CHUNK = 512                          # rows per DMA chunk
N_CHUNKS = N_CORE // CHUNK           # 8
TPC = CHUNK // P                     # 4 row-tiles per chunk
N_TILES = N_CORE // P                # 32
E_HALF = 512                         # fp32 free elems per PSUM bank

_CACHE = {}


def _build_program(n_core_rows, repeats=1):
    import concourse.tile as tile
    from concourse import bacc, mybir

    f16 = mybir.dt.float16
    f8 = mybir.dt.float8e4
    f32 = mybir.dt.float32

    n_chunks = n_core_rows // CHUNK
    n_tiles = n_core_rows // P

    nc = bacc.Bacc("TRN2", target_bir_lowering=False, debug=False)
    elg16p = nc.dram_tensor(
        "elg16p", [n_chunks, P, KF, CHUNK], f16, kind="ExternalInput").ap()
    elg8p = nc.dram_tensor(
        "elg8p", [n_chunks, P, KQ, 2, CHUNK], f8, kind="ExternalInput").ap()
    eth16p = nc.dram_tensor(
        "eth16p", [n_chunks, P, TPC, D], f16, kind="ExternalInput").ap()
    w16p = nc.dram_tensor("w16p", [P, KF, D], f16, kind="ExternalInput").ap()
    w8p = nc.dram_tensor("w8p", [P, KQ, 2, D], f8, kind="ExternalInput").ap()
    out = nc.dram_tensor(
        "out", [P, n_tiles * repeats], f32, kind="ExternalOutput").ap()

    with tile.TileContext(nc) as tc:
        with tc.tile_pool(name="w_pool", bufs=1) as w_pool, \
             tc.tile_pool(name="lg16_pool", bufs=2) as lg16_pool, \
             tc.tile_pool(name="lg8_pool", bufs=2) as lg8_pool, \
             tc.tile_pool(name="et_pool", bufs=2) as et_pool, \
             tc.tile_pool(name="pr_pool", bufs=2) as pr_pool, \
             tc.tile_pool(name="acc_pool", bufs=1) as acc_pool, \
             tc.tile_pool(name="ps_pool", bufs=3, space="PSUM") as ps_pool:

            w16_sb = w_pool.tile([P, KF, D], f16, name="w16_sb")
            w8_sb = w_pool.tile([P, KQ, 2, D], f8, name="w8_sb")
            # k-separate DMAs so the first matmul group's weights land first
            for k in range(KF):
                nc.sync.dma_start(out=w16_sb[:, k, :], in_=w16p[:, k, :])
            for k in range(KQ):
                nc.sync.dma_start(out=w8_sb[:, k, :, :], in_=w8p[:, k, :, :])

            out_sb = acc_pool.tile([P, n_tiles * repeats], f32, name="out_sb")

            for _rep in range(repeats):
                t_idx = _rep * n_tiles
                for j in range(n_chunks):
                    lg16 = lg16_pool.tile([P, KF, CHUNK], f16, name="lg16")
                    nc.sync.dma_start(out=lg16[:], in_=elg16p[j])
                    lg8 = lg8_pool.tile([P, KQ, 2, CHUNK], f8, name="lg8")
                    nc.sync.dma_start(out=lg8[:], in_=elg8p[j])
                    et = et_pool.tile([P, TPC, D], f16, name="et")
                    nc.sync.dma_start(out=et[:], in_=eth16p[j])

                    for s in range(TPC):
                        t_ps = ps_pool.tile([P, D], f32, name="t_ps")
                        for eh in range(2):
                            pcols = t_ps[:, eh * E_HALF:(eh + 1) * E_HALF]
                            for k in range(KF):
                                nc.tensor.matmul(
                                    pcols,
                                    lg16[:, k, s * P:(s + 1) * P],
                                    w16_sb[:, k, eh * E_HALF:(eh + 1) * E_HALF],
                                    start=(k == 0),
                                    stop=False,
                                )
                            for k in range(KQ):
                                nc.tensor.matmul(
                                    pcols,
                                    lg8[:, k, :, s * P:(s + 1) * P],
                                    w8_sb[:, k, :, eh * E_HALF:(eh + 1) * E_HALF],
                                    start=False,
                                    stop=(k == KQ - 1),
                                    perf_mode=mybir.MatmulPerfMode.DoubleRow,
                                )
                        prod = pr_pool.tile([P, D], f32, name="prod")
                        nc.vector.affine_mul_reduce(
                            out=prod[:],
                            accum_out=out_sb[:, t_idx:t_idx + 1],
                            in0=t_ps[:],
                            in1=et[:, s, :],
                            scale=1.0,
                            bias=0.0,
                        )
                        t_idx += 1

            nc.sync.dma_start(out=out, in_=out_sb[:])

    nc.compile()
    return nc


def _make_runner(nc, n_cores):
    """Mirror bass2jax.run_bass_via_pjrt's multi-core branch, but return a
    cached jitted callable so repeat calls skip retracing.
    """
    import jax
    import concourse.mybir as mybir
    from concourse import bass2jax
    from jax.experimental.shard_map import shard_map
    from jax.sharding import Mesh, PartitionSpec

    bass2jax.install_neuronx_cc_hook()
    assert nc.dbg_addr is None
    partition_name = nc.partition_id_tensor.name if nc.partition_id_tensor else None

    in_names, out_names, out_avals = [], [], []
    for alloc in nc.m.functions[0].allocations:
        if not isinstance(alloc, mybir.MemoryLocationSet):
            continue
        name = alloc.memorylocations[0].name
        if alloc.kind == "ExternalInput":
            if name != partition_name:
                in_names.append(name)
        elif alloc.kind == "ExternalOutput":
            shape = tuple(alloc.tensor_shape)
            dtype = mybir.dt.np(alloc.dtype)
            out_names.append(name)
            out_avals.append(jax.core.ShapedArray(shape, dtype))
    n_params = len(in_names)
    n_outs = len(out_avals)
    all_in_names = in_names + out_names
    if partition_name is not None:
        all_in_names = all_in_names + [partition_name]

    def _body(*args):
        operands = list(args)
        if partition_name is not None:
            operands.append(bass2jax.partition_id_tensor())
        outs = bass2jax._bass_exec_p.bind(
            *operands,
            out_avals=tuple(out_avals),
            in_names=tuple(all_in_names),
            out_names=tuple(out_names),
            lowering_input_output_aliases=(),
            sim_require_finite=True,
            sim_require_nnan=True,
            nc=nc,
        )
        return tuple(outs)

    devices = jax.devices()[:n_cores]
    assert len(devices) == n_cores
    mesh = Mesh(np.asarray(devices), ("core",))
    spec = PartitionSpec("core")
    sharded = jax.jit(
        shard_map(
            _body,
            mesh=mesh,
            in_specs=(spec,) * (n_params + n_outs),
            out_specs=(spec,) * n_outs,
            check_rep=False,
        ),
        donate_argnums=tuple(range(n_params, n_params + n_outs)),
        keep_unused=True,
    )
    zero_out_shapes = [
        ((n_cores * av.shape[0],) + tuple(av.shape[1:]), av.dtype) for av in out_avals
    ]
    return sharded, in_names, out_names, zero_out_shapes, mesh, spec


def _get_runner():
    r = _CACHE.get("runner")
    if r is None:
        nc = _build_program(N_CORE)
        r = _CACHE["runner"] = _make_runner(nc, N_CORES)
    return r


def _global_inputs(elg, eth, weight):
    """Host-side marshalling: cast + pack the global arrays per-core.

    elg16p [8*NCH, P, KF, CHUNK] f16 : elg16p[c,j,p,k,n] = elg[c*4096+j*CH+n, k*128+p]
    elg8p  [8*NCH, P, KQ, 2, CHUNK] e4m3, slot-duplicated, d >= 512
    eth16p [8*NCH, P, TPC, D] f16
    w16p   [8*P, KF, D] f16 (replicated), w8p [8*P, KQ, 2, D] e4m3 hi/lo pair
    """
    import ml_dtypes

    e4 = ml_dtypes.float8_e4m3fn
    elg16 = elg[:, :D_SPLIT].astype(np.float16)
    elg16p = np.ascontiguousarray(
        elg16.reshape(N_CORES * N_CHUNKS, CHUNK, KF, P).transpose(0, 3, 2, 1)
    )
    s8 = elg[:, D_SPLIT:].astype(e4)
    t8 = s8.reshape(N_CORES * N_CHUNKS, CHUNK, KQ, P).transpose(0, 3, 2, 1)
    elg8p = np.ascontiguousarray(
        np.broadcast_to(t8[:, :, :, None, :], t8.shape[:3] + (2, CHUNK))
    )
    eth16p = np.ascontiguousarray(
        eth.astype(np.float16)
        .reshape(N_CORES * N_CHUNKS, TPC, P, D)
        .transpose(0, 2, 1, 3)
    )
    w16p = np.ascontiguousarray(
        weight[:D_SPLIT].astype(np.float16).reshape(KF, P, D).transpose(1, 0, 2)
    )
    wh = weight[D_SPLIT:].astype(e4)
    wl = (weight[D_SPLIT:] - wh.astype(np.float32)).astype(e4)
    w8 = np.stack([wh, wl], axis=1)                        # [512, 2, D]
    w8p = np.ascontiguousarray(w8.reshape(KQ, P, 2, D).transpose(1, 0, 2, 3))
    w16p_g = np.ascontiguousarray(
        np.broadcast_to(w16p, (N_CORES,) + w16p.shape)
    ).reshape(N_CORES * P, KF, D)
    w8p_g = np.ascontiguousarray(
        np.broadcast_to(w8p, (N_CORES,) + w8p.shape)
    ).reshape(N_CORES * P, KQ, 2, D)
    return {
        "elg16p": elg16p,
        "elg8p": elg8p,
        "eth16p": eth16p,
        "w16p": w16p_g,
        "w8p": w8p_g,
    }


def _core0_inputs(gins):
    """Per-core-0 slice of the global packed inputs (for 1-core programs)."""
    return {
        "elg16p": gins["elg16p"][:N_CHUNKS],
        "elg8p": gins["elg8p"][:N_CHUNKS],
        "eth16p": gins["eth16p"][:N_CHUNKS],
        "w16p": gins["w16p"][:P],
        "w8p": gins["w8p"][:P],
    }


def _call_runner(global_ins):
    sharded, in_names, out_names, zero_out_shapes, _, _ = _get_runner()
    zeros = [np.zeros(shape, dt) for shape, dt in zero_out_shapes]
    out_arrs = sharded(*[global_ins[n] for n in in_names], *zeros)
    out_g = np.asarray(out_arrs[out_names.index("out")])  # [8*128, 32]
    return np.concatenate(
        [out_g[c * P:(c + 1) * P].T.reshape(-1) for c in range(N_CORES)]
    ).astype(np.float32)


def kernel(elg, eth, weight):
    elg = np.asarray(elg, dtype=np.float32)
    eth = np.asarray(eth, dtype=np.float32)
    weight = np.asarray(weight, dtype=np.float32)
    return _call_runner(_global_inputs(elg, eth, weight))
